# revision 19
# baseline (speedup 1.0000x reference)
"""Causal self-attention (B=2, T=2048, D=1024, H=16, Dh=64) on 8 TRN2 cores.

Sharding: core c = 4*b + g -> batch b (data parallel), head group g of 4
heads (tensor parallel on heads for Wq/Wk/Wv, column-split of the proj
input with the resulting partial-sum reduction done host-side at unshard).

Per-core dataflow (layouts chosen so no on-device transposes are needed):
  qT,kT [256, 2048] bf16 = W{q,k}_g @ x.T   (lhsT = W{q,k}_g.T from host)
  v     [t-block 128, 4 heads x (64 v | 64 ones)] bf16
  attention runs per head PAIR (the two heads sharing a qT/kT tile's
  128 partitions) over 512-wide q chunks:
    PT[tk, 2, tq] = two concurrent 64-row-tiled matmuls (tile_position
    (0,0) and (64,0) stream simultaneously through the PE array, one
    PSUM bank each)
    exp on ACT covers both heads in ONE ACTIVATE (strided AP), bf16 out
    causal mask as post-exp 0/1 multiply on GPSIMD (diag blocks only)
    AV: yT[d, 2, tq] accumulates per head; softmax column sums come free
    via the ones columns of v
  normalize: 1/sums via the custom-DVE reciprocal_approx_fast (reads the
  PSUM accumulator directly), then yT * rc -> ytsb [128, 2048] f32r
  proj partial: out[t, :] = ytsb.T-block @ Wp_gT  (f32r)
Host: out[b] = sum_g partial[4b+g] + bp.

QKV projections, v-blocks and output-projection blocks are drip-fed as PE
filler work inside the (ACT-paced) attention chunks so the PE never idles
and the HAM clock gate stays at 8/8.
"""

import numpy as np

import concourse.bass as bass
import concourse.mybir as mybir
import concourse.tile as tile
from concourse import bacc
from concourse import bass_utils

F32 = mybir.dt.float32
F32R = mybir.dt.float32r
BF16 = mybir.dt.bfloat16

B, T, D = 2, 2048, 1024
H, DH = 16, 64
N_CORES = 8
HPC = 4            # heads per core
GD = HPC * DH      # 256 feature cols per core
KT = D // 128      # 8 k-tiles over the model dim
TB = T // 128      # 16 t-blocks of 128
SCL = 0.125        # logit scale 1/sqrt(Dh)

_cache = {}


def _build():
    nc = bacc.Bacc("TRN2", target_bir_lowering=False, debug=False,
                   num_devices=N_CORES)

    x4_d = nc.dram_tensor("x4", [4, 128, KT, 512], BF16, kind="ExternalInput")
    wqT_d = nc.dram_tensor("wqT", [128, KT, GD], BF16, kind="ExternalInput")
    wkT_d = nc.dram_tensor("wkT", [128, KT, GD], BF16, kind="ExternalInput")
    wvT_d = nc.dram_tensor("wvT", [128, KT, GD], BF16, kind="ExternalInput")
    wpT_d = nc.dram_tensor("wpT", [GD, D], F32R, kind="ExternalInput")
    bq_d = nc.dram_tensor("bq2", [128, 2], F32, kind="ExternalInput")
    bk_d = nc.dram_tensor("bk2", [128, 2], F32, kind="ExternalInput")
    bvb_d = nc.dram_tensor("bvb", [128, GD], F32, kind="ExternalInput")
    msk_d = nc.dram_tensor("mask01", [128, 128], BF16, kind="ExternalInput")
    out_d = nc.dram_tensor("out", [T, D], F32, kind="ExternalOutput")
    wrm_d = nc.dram_tensor("wrm", [2, 1], BF16, kind="ExternalOutput")

    with tile.TileContext(nc) as tc:
        with (
            tc.tile_pool(name="const", bufs=1) as cp,
            tc.tile_pool(name="big", bufs=1) as bp_,
            tc.tile_pool(name="work", bufs=4) as wp_,
            tc.tile_pool(name="outp", bufs=6) as op_,
            tc.tile_pool(name="pA", bufs=2, space="PSUM") as pA,
            tc.tile_pool(name="pB", bufs=1, space="PSUM") as pB,
            tc.tile_pool(name="pC", bufs=2, space="PSUM") as pC,
        ):
            # ---- loads. The prologue is aggregate-DMA-bound (~5MB of
            # inputs), so: tiny tiles first (mask feeds the PE warm-up),
            # then wq + all of x (the gate for the first attention chunk)
            # striped over three queues, then wk/wv, wpt dead last. The
            # scalar queue carries no DMAs at all - enqueues there would
            # steal issue slots from the (near-critical) ACTIVATE stream. ----
            msk = cp.tile([128, 128], BF16, tag="msk", name="msk")
            bq2 = cp.tile([128, 2], F32, tag="bq2", name="bq2")
            bk2 = cp.tile([128, 2], F32, tag="bk2", name="bk2")
            bvb = cp.tile([128, GD], F32, tag="bvb", name="bvb")
            # x loads in four column-chunks: attention chunk ci only needs
            # x columns [512ci, 512ci+512) for its q/k/v, so the first
            # attention chunk starts after 1MB of x instead of 4MB; later
            # chunks pipeline behind the remaining transfers.
            nc.gpsimd.dma_start(msk[:], msk_d[:])
            wq = cp.tile([128, KT, GD], BF16, tag="wq", name="wq")
            wk = cp.tile([128, KT, GD], BF16, tag="wk", name="wk")
            wv = cp.tile([128, KT, GD], BF16, tag="wv", name="wv")
            xn = []
            for n in range(4):
                t_ = cp.tile([128, KT, 512], BF16, tag=f"xn{n}", name=f"xn{n}")
                xn.append(t_)
            # critical prefix wq+wk+xn0 (2MB) spread evenly over the three
            # DMA queues; everything else lands just-in-time behind it
            nc.sync.dma_start(bq2[:], bq_d[:])
            nc.sync.dma_start(bk2[:], bk_d[:])
            nc.sync.dma_start(xn[0][:, 0:4, :], x4_d[0][:, 0:4, :])
            nc.scalar.dma_start(xn[0][:, 4:8, :], x4_d[0][:, 4:8, :])
            nc.gpsimd.dma_start(wq[:], wqT_d[:])
            nc.sync.dma_start(wk[:, 0:4, :], wkT_d[:, 0:4, :])
            nc.gpsimd.dma_start(wk[:, 4:8, :], wkT_d[:, 4:8, :])
            nc.sync.dma_start(xn[1][:, 0:4, :], x4_d[1][:, 0:4, :])
            nc.scalar.dma_start(xn[1][:, 4:8, :], x4_d[1][:, 4:8, :])
            nc.scalar.dma_start(wv[:], wvT_d[:])
            nc.gpsimd.dma_start(xn[2][:, 0:4, :], x4_d[2][:, 0:4, :])
            nc.scalar.dma_start(xn[2][:, 4:8, :], x4_d[2][:, 4:8, :])
            nc.sync.dma_start(xn[3][:, 0:4, :], x4_d[3][:, 0:4, :])
            nc.gpsimd.dma_start(xn[3][:, 4:8, :], x4_d[3][:, 4:8, :])
            nc.sync.dma_start(bvb[:], bvb_d[:])
            wpt = []
            for p in range(2):
                t_ = cp.tile([128, D], F32R, tag=f"wp{p}", name=f"wp{p}")
                (nc.sync, nc.gpsimd)[p].dma_start(
                    t_[:], wpT_d[p * 128:(p + 1) * 128, :])
                wpt.append(t_)

            def xs(k, c0, c1):
                n = c0 // 512
                return xn[n][:, k, c0 - 512 * n:c1 - 512 * n]

            # preload the exp table set while the DMAs run
            wrm = wp_.tile([128, 1], BF16, tag="wrm", name="wrm", bufs=1)
            nc.scalar.activation(wrm[:], bq2[:, 0:1],
                                 mybir.ActivationFunctionType.Exp, scale=0.01)

            nc.gpsimd.dma_start(wrm_d[:], wrm[0:2, :])

            qt = [bp_.tile([128, T], BF16, tag=f"qt{m}", name=f"qt{m}")
                  for m in range(2)]
            kt = [bp_.tile([128, T], BF16, tag=f"kt{m}", name=f"kt{m}")
                  for m in range(2)]
            ytsb = [bp_.tile([128, T], F32R, tag=f"yt{p}", name=f"yt{p}")
                    for p in range(2)]
            vt = [bp_.tile([128, 4, 2, DH], BF16, tag=f"v{t}", name=f"v{t}")
                  for t in range(TB)]

            def qk_group(dst, w, b2, m, n):
                ps = pC.tile([128, 512], F32, tag=pC.name, name="psqk")
                for k in reversed(range(KT)):
                    nc.tensor.matmul(
                        ps[:],
                        w[:, k, m * 128:(m + 1) * 128],
                        xs(k, n * 512, (n + 1) * 512),
                        start=(k == KT - 1), stop=(k == 0),
                    )
                nc.vector.tensor_scalar_add(
                    dst[m][:, n * 512:(n + 1) * 512], ps[:], b2[:, m:m + 1],
                )

            def v_group(t):
                nc.gpsimd.memset(vt[t][:, :, 1, :], 1.0)
                ps = pC.tile([128, 512], F32, tag=pC.name, name="psv")
                for k in reversed(range(KT)):
                    nc.tensor.matmul(
                        ps[:, 0:GD],
                        xs(k, t * 128, (t + 1) * 128),
                        wv[:, k, :],
                        start=(k == KT - 1), stop=(k == 0),
                    )
                nc.vector.tensor_add(
                    vt[t][:, :, 0, :],
                    ps[:, 0:GD].rearrange("p (h d) -> p h d", h=4),
                    bvb.rearrange("p (h d) -> p h d", h=4),
                )

            obs = {}

            def proj_half(t, n, tail=False):
                if n == 0:
                    obs[t] = op_.tile([128, 1024], F32, tag="ob", name="ob")
                ob = obs[t]
                po = pC.tile([128, 512], F32, tag=pC.name, name="pso")
                for p in range(2):
                    nc.tensor.matmul(
                        po[:],
                        ytsb[p][:, 128 * t:128 * (t + 1)],
                        wpt[p][:, 512 * n:512 * (n + 1)],
                        start=(p == 0), stop=(p == 1),
                    )
                if tail and n == 0:
                    nc.scalar.copy(ob[:, 512 * n:512 * (n + 1)], po[:])
                else:
                    nc.vector.tensor_copy(ob[:, 512 * n:512 * (n + 1)], po[:])
                if tail:
                    eng = (nc.sync, nc.gpsimd)[(2 * t + n) % 2]
                    eng.dma_start(
                        out_d[128 * t:128 * (t + 1), 512 * n:512 * (n + 1)],
                        ob[:, 512 * n:512 * (n + 1)])
                    if n == 1:
                        del obs[t]
                elif n == 1:
                    eng = (nc.sync, nc.scalar)[t % 2]
                    eng.dma_start(out_d[128 * t:128 * (t + 1), :], ob[:])
                    del obs[t]

            def proj_group(t, tail=False):
                proj_half(t, 0, tail)
                proj_half(t, 1, tail)

            def att_chunk(hp, ci, fillers, every, last=False):
                """q-chunk [512ci, 512ci+512) for BOTH heads of pair hp.

                The two heads' QK matmuls go to PE row tiles (0,0)/(64,0)
                back-to-back so they stream concurrently; their logits share
                one [128, 2, 512] PSUM tile (one bank per head), covered by
                a single exp ACTIVATE."""
                nblk = 4 * (ci + 1)
                SKEW = 3
                q0 = 512 * ci
                ytp = pB.tile([128, 2, 512], F32, tag=pB.name, name="ytp")
                pend = []

                def do_av(t, ptsb):
                    s = max(0, 128 * t - q0)
                    for j in range(2):
                        nc.tensor.matmul(
                            ytp[:, j, s:512],
                            vt[t][:, 2 * hp + j, :, :].rearrange(
                                "p a d -> p (a d)"),
                            ptsb[:, j, s:512],
                            start=(t == 0), stop=(t == nblk - 1),
                        )

                for t in range(nblk + SKEW):
                    if fillers and t % every == every - 1:
                        fillers.pop(0)()
                    if t < nblk:
                        s = max(0, 128 * t - q0)
                        pt = pA.tile([128, 2, 512], F32, tag=pA.name,
                                     name="pt")
                        for j in range(2):
                            nc.tensor.matmul(
                                pt[:, j, s:512],
                                kt[hp][64 * j:64 * j + 64,
                                       128 * t:128 * (t + 1)],
                                qt[hp][64 * j:64 * j + 64,
                                       q0 + s:q0 + 512],
                                start=True, stop=True,
                            )
                        ptsb = wp_.tile([128, 2, 512], BF16, tag="ptsb",
                                        name="ptsb", bufs=6)
                        nc.scalar.activation(
                            ptsb[:, :, s:512], pt[:, :, s:512],
                            mybir.ActivationFunctionType.Exp, scale=SCL,
                        )
                        if 128 * t >= q0:  # diagonal block
                            for j in range(2):
                                nc.gpsimd.tensor_mul(
                                    ptsb[:, j, s:s + 128],
                                    ptsb[:, j, s:s + 128], msk[:])
                        pend.append((t, ptsb))
                    if t >= SKEW:
                        do_av(*pend.pop(0))
                while pend:
                    do_av(*pend.pop(0))
                # normalize: sums sit on partitions 64-127 (the ones columns
                # of v), replicated down the 64 partitions so the elementwise
                # multiply broadcasts for free. reciprocal_approx_fast needs
                # a base-partition-0 SBUF input (PSUM or partition-offset
                # inputs return garbage), so stage the sums through SBUF.
                # Per-head chains so the ytp banks free up one at a time and
                # the next chunk's first AV isn't gated on the whole chain.
                nh = 2 if (hp == 1 or last) else 1
                w = 512 // nh
                for h in range(nh):
                    for j in range(2):
                        scp = wp_.tile([64, 512], F32, tag="scp", name="scp")
                        nc.vector.tensor_copy(
                            scp[:, 0:w], ytp[64:128, j, h * w:(h + 1) * w])
                        rc = wp_.tile([64, 512], F32, tag="recip", name="recip")
                        nc.vector.reciprocal_approx_fast(rc[:, 0:w], scp[:, 0:w])
                        nc.vector.tensor_mul(
                            ytsb[hp][64 * j:64 * j + 64,
                                     q0 + h * w:q0 + (h + 1) * w],
                            ytp[0:64, j, h * w:(h + 1) * w], rc[:, 0:w],
                        )

            # ---- schedule: only the first qk n-chunk runs before
            # attention; everything else drips into the (ACT-paced)
            # attention chunks as PE filler groups ----
            qk_group(qt, wq, bq2, 0, 0)
            qk_group(kt, wk, bk2, 0, 0)
            v_group(0)
            v_group(1)

            att_chunk(0, 0, [
                lambda: v_group(2), lambda: v_group(3),
                lambda: qk_group(qt, wq, bq2, 0, 1),
                lambda: qk_group(kt, wk, bk2, 0, 1),
            ], 1)
            att_chunk(0, 1, [
                lambda: v_group(4), lambda: v_group(5),
                lambda: v_group(6), lambda: v_group(7),
                lambda: qk_group(qt, wq, bq2, 0, 2),
                lambda: qk_group(kt, wk, bk2, 0, 2),
            ], 1)
            att_chunk(0, 2, [
                lambda: v_group(8), lambda: v_group(9),
                lambda: v_group(10), lambda: v_group(11),
                lambda: qk_group(qt, wq, bq2, 0, 3),
                lambda: qk_group(kt, wk, bk2, 0, 3),
            ], 2)
            att_chunk(0, 3, [
                lambda: v_group(12), lambda: v_group(13),
                lambda: v_group(14), lambda: v_group(15),
                lambda: qk_group(qt, wq, bq2, 1, 0),
                lambda: qk_group(kt, wk, bk2, 1, 0),
                lambda: qk_group(qt, wq, bq2, 1, 1),
                lambda: qk_group(kt, wk, bk2, 1, 1),
            ], 2)
            att_chunk(1, 0, [
                lambda: qk_group(qt, wq, bq2, 1, 2),
                lambda: qk_group(kt, wk, bk2, 1, 2),
            ], 2)
            att_chunk(1, 1, [
                lambda: qk_group(qt, wq, bq2, 1, 3),
                lambda: qk_group(kt, wk, bk2, 1, 3),
            ] + [lambda t=t, n=n: proj_half(t, n)
                 for t in range(0, 2) for n in range(2)], 1)
            att_chunk(1, 2, [lambda t=t, n=n: proj_half(t, n)
                             for t in range(2, 7) for n in range(2)], 1)
            att_chunk(1, 3, [lambda t=t, n=n: proj_half(t, n)
                             for t in range(7, 12) for n in range(2)], 1,
                      last=True)
            for t in range(12, 16):
                proj_group(t, tail=True)

    nc.compile()
    return nc


def _shard(x, Wq, bq, Wk, bk, Wv, bv, Wp, bp):
    import ml_dtypes
    f32 = np.float32
    bf16 = ml_dtypes.bfloat16
    mask01 = np.triu(np.ones((128, 128), f32)).astype(bf16)
    in_maps = []
    for c in range(N_CORES):
        b, g = divmod(c, HPC)
        sl = slice(GD * g, GD * (g + 1))
        in_maps.append({
            "x4": np.ascontiguousarray(
                x[b].T.reshape(8, 128, 4, 512).transpose(2, 1, 0, 3)
            ).astype(bf16),
            "wqT": np.ascontiguousarray(
                Wq[sl, :].T.reshape(8, 128, 256).transpose(1, 0, 2)).astype(bf16),
            "wkT": np.ascontiguousarray(
                Wk[sl, :].T.reshape(8, 128, 256).transpose(1, 0, 2)).astype(bf16),
            "wvT": np.ascontiguousarray(
                Wv[sl, :].T.reshape(8, 128, 256).transpose(1, 0, 2)).astype(bf16),
            "wpT": np.ascontiguousarray(Wp[:, sl].T, dtype=f32),
            "bq2": np.ascontiguousarray(bq[sl].reshape(2, 128).T, dtype=f32),
            "bk2": np.ascontiguousarray(bk[sl].reshape(2, 128).T, dtype=f32),
            "bvb": np.broadcast_to(bv[sl], (128, GD)).astype(f32),
            "mask01": mask01,
        })
    return in_maps


def run(inputs, trace=False):
    """Run the SPMD kernel; returns (output [B,T,D] f32, BassKernelResults)."""
    if "nc" not in _cache:
        _cache["nc"] = _build()
    nc = _cache["nc"]
    in_maps = _shard(**inputs)
    if trace:
        _install_ntff_hook()
    res = bass_utils.run_bass_kernel_spmd(
        nc, in_maps, core_ids=list(range(N_CORES)), trace=trace,
    )
    bp = np.asarray(inputs["bp"], dtype=np.float32)
    out = np.empty((B, T, D), dtype=np.float32)
    for b in range(B):
        acc = res.results[4 * b]["out"].astype(np.float32)
        for g in range(1, HPC):
            acc = acc + res.results[4 * b + g]["out"]
        out[b] = acc + bp
    return out, res


def kernel(**inputs):
    out, _ = run(inputs, trace=False)
    return out


def _install_ntff_hook():
    """antenv.axon_hooks is absent on this image; inject it so
    run_bass_kernel_spmd(trace=True) can capture NTFF profiles."""
    import sys, types
    if "antenv.axon_hooks" in sys.modules:
        return
    try:
        mod = types.ModuleType("antenv.axon_hooks")
        mod._hook = None
        mod.set_axon_ntff_profile_hook = lambda h: setattr(mod, "_hook", h)
        mod.get_axon_ntff_profile_hook = lambda: mod._hook
        sys.modules["antenv.axon_hooks"] = mod
        import antenv
        antenv.axon_hooks = mod
        from trn_agent_boot.trn_boot import _ntff_profile_via_ctypes
        mod.set_axon_ntff_profile_hook(
            _ntff_profile_via_ctypes("/opt/axon/libaxon_pjrt.so"))
    except Exception:
        pass
